# revision 1
# baseline (speedup 1.0000x reference)
"""Lattice gauge CNN (L-CNN) layer on 8 TRN2 NeuronCores via Bass.

Self-contained: host packing + device kernel + unpacking. Data-parallel over
batch (B=8 -> one full 12^4 lattice per core).

Device model (per core):
  flat site s = ((a0*12+a1)*12+a2)*12+a3;  p = s % 128;  fs = s // 128 (0..161)
  site-major tiles [128, nch*162] bf16, f = ch*162 + fs  (ch-outer, fs-inner)
  U channels: ch = mu*18 + (3a+c)*2 + rho.
"""
import numpy as np

L = 12
NS = L ** 4            # 20736
NFS = NS // 128        # 162
CHUNK_FS = 18          # 9 chunks x 18 fs
NCHUNK = NFS // CHUNK_FS
CHS = CHUNK_FS * 128   # sites per chunk = 2304
WIN = 384              # MM window; 2304/384 = 6 windows per chunk
NWIN = CHS // WIN
NCH_U = 72
SIG = (1728, 144, 12, 1)


def _roll_plan(mu, delta):
    """Main global-shift over ALL sites (wrap sites get wrong values),
    then overwrite wrap set with corrected shift.  Order matters."""
    sig = SIG[mu]
    if delta > 6:
        delta = delta - 12          # roll +8 == roll -4 (period 12)
    D = delta * sig
    g_map = (np.arange(NS) + D) % NS
    ax = (np.arange(NS) // sig) % 12
    if delta > 0:
        wrapm = ax >= 12 - delta
        fix_map = (np.arange(NS) + D - 12 * sig) % NS
        jset = range(12 - delta, 12)
    else:
        wrapm = ax < -delta
        fix_map = (np.arange(NS) + D + 12 * sig) % NS
        jset = range(0, -delta)
    main = _plan_pieces(g_map, None)
    if sig == 1:
        fix = []
        for j in jset:
            fix += _plan_pieces(fix_map, ax == j)
    else:
        fix = _plan_pieces(fix_map, wrapm)
    return main + fix

WPITCH = 184
W_SLACK = 80

_COMPILED = None


# --------------------------------------------------------------------------
# compile-time site maps
# --------------------------------------------------------------------------
def _site_shift_map(mu, delta):
    idx = np.arange(NS).reshape(L, L, L, L)
    return np.roll(idx, -delta, axis=mu).reshape(-1)


def _col_groups(src_map, w0, n):
    """Decompose shifted-window read into (dst0, src0, step, nblk, ln)."""
    src = src_map[w0:w0 + n]
    runs, st = [], 0
    for i in range(1, n + 1):
        if i == n or src[i] != src[i - 1] + 1:
            runs.append((st, int(src[st]), i - st))
            st = i
    from collections import defaultdict
    byln = defaultdict(list)
    for (d0, s0, ln) in runs:
        byln[ln].append((d0, s0))
    groups = []
    for ln, items in byln.items():
        items.sort()
        i = 0
        while i < len(items):
            d0, s0 = items[i]
            j, step = i + 1, 0
            while j < len(items):
                dd, ss = items[j]
                st2 = dd - items[j - 1][0]
                if ss - items[j - 1][1] != st2:
                    break
                if j == i + 1:
                    step = st2
                elif st2 != step:
                    break
                j += 1
            groups.append((d0, s0, step if j - i > 1 else 0, j - i, ln))
            i = j
    return groups


def _plan_pieces(mp, sel=None):
    """Copy pieces for dst[s] = src[mp[s]] (s in sel or all):
    (d0, s0, pstep, nblk, ln, fstep128, nf)."""
    diff = mp - np.arange(NS)
    if sel is None:
        sel = np.ones(NS, bool)
    raw, st = [], None
    for i in range(NS + 1):
        brk = (i == NS or not sel[i] or
               (st is not None and diff[i] != diff[st]))
        if st is not None and brk:
            D = int(diff[st])
            s = st
            while s < i:
                e = min(i, (s // 128 + 1) * 128, s + 128 - ((s + D) % 128))
                raw.append((s, s + D, e - s))
                s = e
            st = None
        if i < NS and sel[i] and st is None:
            st = i
        elif i < NS and sel[i] and diff[i] != diff[st]:
            st = i
    # level 1: arithmetic progressions (same len, same step in dst and src)
    g1, i = [], 0
    while i < len(raw):
        d0, s0, ln = raw[i]
        j, step = i + 1, 0
        while j < len(raw):
            dd, ss, ll = raw[j]
            if ll != ln:
                break
            st2 = dd - raw[j - 1][0]
            if ss - raw[j - 1][1] != st2:
                break
            # partition windows must not wrap within the progression
            if j == i + 1:
                step = st2
            elif st2 != step:
                break
            if (dd % 128) + ll > 128 or (ss % 128) + ll > 128:
                break
            if ((dd % 128) < (raw[j - 1][0] % 128)
                    or (ss % 128) < (raw[j - 1][1] % 128)):
                break
            j += 1
        g1.append((d0, s0, step if j - i > 1 else 0, j - i, ln))
        i = j
    # level 2: bucket by (p-coords, shape), then uniform-delta families
    from collections import defaultdict
    buckets = defaultdict(list)
    for (d0, s0, stp, nb, ln) in g1:
        buckets[(d0 % 128, s0 % 128, stp, nb, ln)].append((d0, s0))
    out = []
    for (pd, ps, stp, nb, ln), items in buckets.items():
        items.sort()
        i = 0
        while i < len(items):
            d0, s0 = items[i]
            j, fstep = i + 1, 0
            while j < len(items):
                dd, ss = items[j]
                df = dd - items[j - 1][0]
                if ss - items[j - 1][1] != df or df % 128 != 0 or df <= 0:
                    break
                if j == i + 1:
                    fstep = df
                elif df != fstep:
                    break
                j += 1
            out.append((d0, s0, stp, nb, ln,
                        fstep // 128 if j - i > 1 else 0, j - i))
            i = j
    return out


# --------------------------------------------------------------------------
# device build
# --------------------------------------------------------------------------
def _build(nc):
    import concourse.mybir as mybir
    from concourse.tile import TileContext

    F32 = mybir.dt.float32
    BF16 = mybir.dt.bfloat16
    MUL = mybir.AluOpType.mult
    ADD = mybir.AluOpType.add
    SUB = mybir.AluOpType.subtract

    u_d = nc.dram_tensor("u", [128, NCH_U, NFS], BF16, kind="ExternalInput")
    ush_d = nc.dram_tensor("ush", [16, 128, 18 * NFS], BF16,
                           kind="ExternalInput")
    lhsA_d = nc.dram_tensor("lhsA", [128, 36 * 32], BF16, kind="ExternalInput")
    lhsB_d = nc.dram_tensor("lhsB", [64, 24 * 32], BF16, kind="ExternalInput")
    o_d = nc.dram_tensor("o", [128, NCHUNK, 144, CHUNK_FS], F32,
                         kind="ExternalOutput")

    def APC(base, off, dims):
        t = type(base)
        ap = [list(base.ap[0])] + [[int(s), int(n)] for s, n in dims]
        return t(base.tensor, base.offset + int(off), ap)

    from concourse.tile import add_dep_helper

    def TT(out, a, b, op):
        return nc.vector.tensor_tensor(out=out, in0=a, in1=b, op=op)

    shift_maps = {(m, k): _site_shift_map(m, k) for m in range(4)
                  for k in (-1, 1)}

    w_dram = nc.dram_tensor("wtmp", [128, NFS * WPITCH + W_SLACK], BF16,
                            kind="Internal")
    with TileContext(nc) as tc:
        with tc.tile_pool(name="const", bufs=1) as cpool, \
             tc.tile_pool(name="sc", bufs=2) as scpool, \
             tc.tile_pool(name="a1p", bufs=3) as a1pool, \
             tc.tile_pool(name="ps", bufs=6, space="PSUM") as pspool:

            u_t = cpool.tile([128, NCH_U * NFS], BF16, tag="u")
            nc.sync.dma_start(
                u_t[:].rearrange("p (c f) -> p c f", c=NCH_U), u_d[:])
            lhsA_t = cpool.tile([128, 36 * 32], BF16, tag="lhsA")
            nc.sync.dma_start(lhsA_t[:], lhsA_d[:])
            lhsB_t = cpool.tile([64, 24 * 32], BF16, tag="lhsB")
            nc.sync.dma_start(lhsB_t[:], lhsB_d[:])

            def rolled_copy(dst, dch0, src, sch0, mu, delta, nch,
                            wait_for=None):
                pitch_d = dst[:].ap[0][0]
                pitch_s = src[:].ap[0][0]
                T = type(dst[:])
                dmas = []
                _orig_dma = nc.sync.dma_start

                def dma(da, sa):
                    inst = _orig_dma(da, sa)
                    dmas.append(inst)
                    for w in (wait_for or []):
                        if w is not None:
                            add_dep_helper(inst.ins, w.ins,
                                           reason="rolled-copy RAW")
                    return inst

                def ap_of(base, pitch, p0, off, dims):
                    return T(base[:].tensor, base[:].offset + p0 * pitch + off,
                             [[pitch, dims[0]]] + dims[1])

                for (d0, s0, pst, nb, ln, fst, nf) in _roll_plan(mu, delta):
                    if nf == 0:
                        nf = 1
                    if (ln == 128 and pst == 128 and d0 % 128 == 0
                            and s0 % 128 == 0 and nf <= 1):
                        for ch in [None]:
                            da = ap_of(dst, pitch_d, 0, dch0 * NFS + d0 // 128,
                                       [128, [[NFS, nch], [1, nb]]])
                            sa = ap_of(src, pitch_s, 0, sch0 * NFS + s0 // 128,
                                       [128, [[NFS, nch], [1, nb]]])
                            dma(da, sa)
                        continue
                    if (ln == 1 and nb > 1 and pst > 0
                            and (d0 % 128) + pst * (nb - 1) < 128
                            and (s0 % 128) + pst * (nb - 1) < 128):
                        if nch <= nf:
                            for ch in range(nch):
                                da = T(dst[:].tensor, dst[:].offset
                                       + (d0 % 128) * pitch_d
                                       + (dch0 + ch) * NFS + d0 // 128,
                                       [[pitch_d * pst, nb], [fst, nf], [1, 1]])
                                sa = T(src[:].tensor, src[:].offset
                                       + (s0 % 128) * pitch_s
                                       + (sch0 + ch) * NFS + s0 // 128,
                                       [[pitch_s * pst, nb], [fst, nf], [1, 1]])
                                dma(da, sa)
                        else:
                            for f in range(nf):
                                da = T(dst[:].tensor, dst[:].offset
                                       + (d0 % 128) * pitch_d
                                       + dch0 * NFS + d0 // 128 + fst * f,
                                       [[pitch_d * pst, nb], [NFS, nch], [1, 1]])
                                sa = T(src[:].tensor, src[:].offset
                                       + (s0 % 128) * pitch_s
                                       + sch0 * NFS + s0 // 128 + fst * f,
                                       [[pitch_s * pst, nb], [NFS, nch], [1, 1]])
                                dma(da, sa)
                        continue
                    for v in range(nb):
                        for f in range(nf):
                            dv = d0 + pst * v + fst * 128 * f
                            sv = s0 + pst * v + fst * 128 * f
                            dp, dfs = dv % 128, dv // 128
                            sp, sfs = sv % 128, sv // 128
                            da = APC(dst[dp:dp + ln, :],
                                     dch0 * NFS + dfs, [[NFS, nch], [1, 1]])
                            sa = APC(src[sp:sp + ln, :],
                                     sch0 * NFS + sfs, [[NFS, nch], [1, 1]])
                            dma(da, sa)
                return dmas

            # complex 3x3 matmul C = A @ B(^dag), full lattice
            def cmat(At, Ach0, Bt, Bch0, writer, adjB=False, pre_deps=None):
                e = NFS
                insts = []
                for a in range(3):
                    P = [scpool.tile([128, 9 * e], BF16, tag=f"pp{t}", name=f"pp{t}")
                         for t in range(4)]
                    for t, (ra, rb) in enumerate(
                            ((0, 0), (1, 1), (0, 1), (1, 0))):
                        a_ap = APC(At[:], (Ach0 + (3 * a) * 2 + ra) * NFS,
                                   [[0, 3], [2 * NFS, 3], [1, e]])
                        if not adjB:
                            bdims = [[2 * NFS, 3], [6 * NFS, 3], [1, e]]
                        else:
                            bdims = [[6 * NFS, 3], [2 * NFS, 3], [1, e]]
                        b_ap = APC(Bt[:], (Bch0 + rb) * NFS, bdims)
                        o_ap = APC(P[t][:], 0, [[3 * e, 3], [e, 3], [1, e]])
                        ii = TT(o_ap, a_ap, b_ap, MUL)
                        if not insts:
                            for d in (pre_deps or []):
                                add_dep_helper(ii.ins, d.ins,
                                               reason="cmat RAW roll")
                        insts.append(ii)
                    f = [APC(P[t][:], 0, [[1, 9 * e]]) for t in range(4)]
                    if not adjB:
                        TT(f[0], f[0], f[1], SUB)
                        TT(f[2], f[2], f[3], ADD)
                        Dr, Di = P[0], P[2]
                    else:
                        TT(f[0], f[0], f[1], ADD)
                        TT(f[3], f[3], f[2], SUB)
                        Dr, Di = P[0], P[3]
                    for rho, D in ((0, Dr), (1, Di)):
                        A1 = a1pool.tile([128, 3 * e], BF16, tag="a1", name="a1")
                        TT(APC(A1[:], 0, [[e, 3], [1, e]]),
                           APC(D[:], 0, [[3 * e, 3], [1, e]]),
                           APC(D[:], e, [[3 * e, 3], [1, e]]), ADD)
                        insts.append(writer(a, rho,
                               APC(A1[:], 0, [[e, 3], [1, e]]),
                               APC(D[:], 2 * e, [[3 * e, 3], [1, e]])))
                return insts[-1]

            def wr_link(tile):
                def w(a, rho, x_ap, y_ap):
                    o = APC(tile[:], ((3 * a) * 2 + rho) * NFS,
                            [[2 * NFS, 3], [1, NFS]])
                    return TT(o, x_ap, y_ap, ADD)
                return w

            # ------------- phase B: W production ------------------------
            with tc.tile_pool(name="wp", bufs=1) as wpool, \
                 tc.tile_pool(name="ush", bufs=3) as ushpool, \
                 tc.tile_pool(name="rp", bufs=3) as rpool, \
                 tc.tile_pool(name="vp", bufs=6) as vpool:
                w_t = wpool.tile([128, NFS * WPITCH + W_SLACK], BF16, tag="w")
                nc.vector.memset(w_t[:], 0.0)

                def wr_W(jw):
                    def w(a, rho, x_ap, y_ap):
                        o = APC(w_t[:], (3 * a) * 20 + jw * 2 + rho,
                                [[20, 3], [WPITCH, NFS]])
                        return TT(o, x_ap, y_ap, ADD)
                    return w

                plaq = 0
                for mu in range(4):
                    for nu in range(mu + 1, 4):
                        u1 = ushpool.tile([128, 18 * NFS], BF16, tag="ush", name="ush1")
                        nc.sync.dma_start(u1[:], ush_d[mu * 4 + nu])
                        u2 = ushpool.tile([128, 18 * NFS], BF16, tag="ush", name="ush2")
                        nc.sync.dma_start(u2[:], ush_d[nu * 4 + mu])
                        Rmn = rpool.tile([128, 18 * NFS], BF16, tag="rt")
                        nc.vector.memset(Rmn[:], 0.0)
                        cmat(u_t, mu * 18, u1, 0, wr_link(Rmn))
                        Rnm = rpool.tile([128, 18 * NFS], BF16, tag="rt")
                        nc.vector.memset(Rnm[:], 0.0)
                        cmat(u_t, nu * 18, u2, 0, wr_link(Rnm))
                        cmat(Rmn, 0, Rnm, 0, wr_W(plaq), adjB=True)
                        plaq += 1
                for mu in range(4):
                    uv = ushpool.tile([128, 18 * NFS], BF16, tag="ush", name="ushv")
                    nc.sync.dma_start(uv[:], ush_d[mu * 4 + mu])
                    V2 = vpool.tile([128, 18 * NFS], BF16, tag="vt")
                    v2w = cmat(u_t, mu * 18, uv, 0, wr_link(V2))
                    V2s = vpool.tile([128, 18 * NFS], BF16, tag="vt")
                    d2 = rolled_copy(V2s, 0, V2, 0, mu, 2, 18, wait_for=[v2w])
                    V4 = vpool.tile([128, 18 * NFS], BF16, tag="vt")
                    v4w = cmat(V2, 0, V2s, 0, wr_link(V4), pre_deps=d2)
                    V4s4 = vpool.tile([128, 18 * NFS], BF16, tag="vt")
                    d4 = rolled_copy(V4s4, 0, V4, 0, mu, 4, 18, wait_for=[v4w])
                    V8 = vpool.tile([128, 18 * NFS], BF16, tag="vt")
                    v8w = cmat(V4, 0, V4s4, 0, wr_link(V8), pre_deps=d4)
                    V4s8 = vpool.tile([128, 18 * NFS], BF16, tag="vt")
                    d8 = rolled_copy(V4s8, 0, V4, 0, mu, 8, 18, wait_for=[v4w])
                    cmat(V8, 0, V4s8, 0, wr_W(6 + mu), pre_deps=d8 + [v8w])

                nc.sync.dma_start(w_dram[:], w_t[:])

            # ---------- phase C: W -> channel-major (xbar from DRAM) ----
            with tc.tile_pool(name="wcmp", bufs=1) as wcmpool, \
                 tc.tile_pool(name="chk", bufs=1) as chkpool, \
                 tc.tile_pool(name="tcm", bufs=2) as tcmpool:
                wcmA = wcmpool.tile([128, NS], BF16, tag="wcmA")
                wcmB = wcmpool.tile([128, NS], BF16, tag="wcmB")
                for fs in range(NFS):
                    nc.sync.dma_start_transpose(
                        wcmA[:, fs * 128:(fs + 1) * 128],
                        w_dram[:, fs * WPITCH:fs * WPITCH + 128])
                    nc.sync.dma_start_transpose(
                        wcmB[:, fs * 128:(fs + 1) * 128],
                        w_dram[:, fs * WPITCH + 120:fs * WPITCH + 248])

                # ---------- phase D: per-chunk T + sandwich -------------
                traw = chkpool.tile([128, CHUNK_FS * 640], BF16, tag="traw")
                t2_t = chkpool.tile([128, CHUNK_FS * 576], BF16, tag="t2")
                x_t = chkpool.tile([128, CHUNK_FS * 144], BF16, tag="x")
                oc_t = chkpool.tile([128, CHUNK_FS * 144], F32, tag="oc")

                e = CHUNK_FS
                for c in range(NCHUNK):
                    fs0 = c * CHUNK_FS
                    for wi in range(NWIN):
                        w0 = c * CHS + wi * WIN
                        pst = {}
                        for g in range(5):
                            ps = pspool.tile([128, WIN], F32, tag="psg", name="psg")
                            pst[g] = ps
                            rows = 128 if g < 3 else 64
                            wcm = wcmA if g < 3 else wcmB
                            for m in range(4):
                                M = 32 if g < 4 else 16
                                if g < 3:
                                    def lh(k):
                                        i0 = ((m * 3 + k) * 3 + g) * 32
                                        return lhsA_t[:, i0:i0 + 32]
                                else:
                                    def lh(k):
                                        i0 = ((m * 3 + k) * 2 + (g - 3)) * 32
                                        return lhsB_t[0:64, i0:i0 + M]
                                po = ps[32 * m:32 * m + M, :]
                                # k=1: zero shift, clears bank slice
                                nc.tensor.matmul(po, lh(1), wcm[0:rows, w0:w0 + WIN],
                                                 start=True, stop=False,
                                                 tile_position=(0, 32 * m))
                                pieces = []
                                for k, kk in ((-1, 0), (1, 2)):
                                    for (d0, s0, stp, nb, ln) in _col_groups(
                                            shift_maps[(m, k)], w0, WIN):
                                        pieces.append((kk, d0, s0, stp, nb, ln))
                                for idx, (kk, d0, s0, stp, nb, ln) in enumerate(pieces):
                                    cdims = ([[stp, nb], [1, ln]] if nb > 1
                                             else [[1, ln]])
                                    o_ap = APC(po, d0, cdims)
                                    r_ap = APC(wcm[0:rows, :], s0, cdims)
                                    nc.tensor.matmul(o_ap, lh(kk), r_ap,
                                                     start=False,
                                                     stop=(idx == len(pieces) - 1),
                                                     tile_position=(0, 32 * m))
                        for g in range(5):
                            tcm = tcmpool.tile([128, WIN], BF16, tag=f"tcm{g}", name=f"tcm{g}")
                            nc.scalar.copy(tcm[:], pst[g][:])
                            for blk in range(WIN // 128):
                                fsl = wi * (WIN // 128) + blk
                                nc.sync.dma_start_transpose(
                                    traw[:, fsl * 640 + g * 128:
                                         fsl * 640 + g * 128 + 128],
                                    tcm[:, blk * 128:(blk + 1) * 128])

                    # repack traw -> t2 (ch-outer, clean cd strides), on ACT
                    for m in range(4):
                        for par in (0, 1):
                            ng = 5 - par
                            for pr in (0, 1):
                                i_ap = APC(traw[:], m * 32 + par * 16 + pr,
                                           [[2, 8], [128, ng], [640, e]])
                                o_ap = APC(t2_t[:], (m * 144 + par * 2 + pr) * e,
                                           [[18 * e, 8], [4 * e, ng], [1, e]])
                                nc.scalar.copy(o_ap, i_ap)

                    # sandwich: out = sum_m U_m @ T_m @ U_m^dag
                    for m in range(4):
                        # stage 1: X[i,a,b] = sum_c U[a,c] T[i,3c+b]
                        for a in range(3):
                            for b in range(3):
                                P = [scpool.tile([128, 24 * e], BF16, tag=f"pp{t}", name=f"sp{t}")
                                     for t in range(4)]
                                for t, (ru, rt) in enumerate(
                                        ((0, 0), (1, 1), (0, 1), (1, 0))):
                                    uap = APC(u_t[:],
                                              (m * 18 + (3 * a) * 2 + ru) * NFS + fs0,
                                              [[0, 8], [2 * NFS, 3], [1, e]])
                                    tap = APC(t2_t[:], (m * 144 + b * 2 + rt) * e,
                                              [[18 * e, 8], [6 * e, 3], [1, e]])
                                    o_ap = APC(P[t][:], 0,
                                               [[3 * e, 8], [e, 3], [1, e]])
                                    TT(o_ap, uap, tap, MUL)
                                f = [APC(P[t][:], 0, [[1, 24 * e]]) for t in range(4)]
                                TT(f[0], f[0], f[1], SUB)
                                TT(f[2], f[2], f[3], ADD)
                                for rho, D in ((0, P[0]), (1, P[2])):
                                    A1 = a1pool.tile([128, 8 * e], BF16, tag="sa1", name="sa1")
                                    TT(APC(A1[:], 0, [[e, 8], [1, e]]),
                                       APC(D[:], 0, [[3 * e, 8], [1, e]]),
                                       APC(D[:], e, [[3 * e, 8], [1, e]]), ADD)
                                    xo = APC(x_t[:], ((a * 3 + b) * 2 + rho) * e,
                                             [[18 * e, 8], [1, e]])
                                    TT(xo, APC(A1[:], 0, [[e, 8], [1, e]]),
                                       APC(D[:], 2 * e, [[3 * e, 8], [1, e]]), ADD)
                        # stage 2: out[i,a,b] += sum_d X[i,a,d] conj(U[b,d])
                        for a in range(3):
                            for b in range(3):
                                P = [scpool.tile([128, 24 * e], BF16, tag=f"pp{t}", name=f"sp{t}")
                                     for t in range(4)]
                                for t, (rx, ru) in enumerate(
                                        ((0, 0), (1, 1), (0, 1), (1, 0))):
                                    xap = APC(x_t[:], ((a * 3) * 2 + rx) * e,
                                              [[18 * e, 8], [2 * e, 3], [1, e]])
                                    uap = APC(u_t[:],
                                              (m * 18 + (3 * b) * 2 + ru) * NFS + fs0,
                                              [[0, 8], [2 * NFS, 3], [1, e]])
                                    o_ap = APC(P[t][:], 0,
                                               [[3 * e, 8], [e, 3], [1, e]])
                                    TT(o_ap, xap, uap, MUL)
                                f = [APC(P[t][:], 0, [[1, 24 * e]]) for t in range(4)]
                                TT(f[0], f[0], f[1], ADD)      # XrUr + XiUi
                                TT(f[3], f[3], f[2], SUB)      # XiUr - XrUi
                                for rho, D in ((0, P[0]), (1, P[3])):
                                    A1 = a1pool.tile([128, 8 * e], BF16, tag="sa1", name="sa1")
                                    TT(APC(A1[:], 0, [[e, 8], [1, e]]),
                                       APC(D[:], 0, [[3 * e, 8], [1, e]]),
                                       APC(D[:], e, [[3 * e, 8], [1, e]]), ADD)
                                    oo = APC(oc_t[:], ((a * 3 + b) * 2 + rho) * e,
                                             [[18 * e, 8], [1, e]])
                                    if m == 0:
                                        TT(oo, APC(A1[:], 0, [[e, 8], [1, e]]),
                                           APC(D[:], 2 * e, [[3 * e, 8], [1, e]]),
                                           ADD)
                                    else:
                                        A2 = a1pool.tile([128, 8 * e], BF16,
                                                         tag="sa2", name="sa2")
                                        TT(APC(A2[:], 0, [[e, 8], [1, e]]),
                                           APC(A1[:], 0, [[e, 8], [1, e]]),
                                           APC(D[:], 2 * e, [[3 * e, 8], [1, e]]),
                                           ADD)
                                        TT(oo, oo, APC(A2[:], 0, [[e, 8], [1, e]]),
                                           ADD)
                    nc.sync.dma_start(o_d[:, c], oc_t[:].rearrange(
                        "p (ch f) -> p ch f", ch=144))
    return nc


# --------------------------------------------------------------------------
# host side
# --------------------------------------------------------------------------
def _pack_u(Ub, bf16):
    """Ub [NS, 4, 3, 3] complex64 -> [128, 72, 162] bf16."""
    arr = Ub.reshape(NFS, 128, 4, 3, 3)
    re = arr.real.transpose(1, 2, 3, 4, 0)     # p mu a c fs
    im = arr.imag.transpose(1, 2, 3, 4, 0)
    dev = np.stack([re, im], axis=4)           # p mu a c rho fs
    return np.ascontiguousarray(dev.reshape(128, NCH_U, NFS)).astype(bf16)


def _pack_ush(Ub, bf16):
    """16 host-shifted link fields -> [16, 128, 18*162] bf16."""
    out = np.empty((16, 128, 18 * NFS), bf16)
    for mu in range(4):
        mp = _site_shift_map(mu, 1)
        for nu in range(4):
            f = Ub[mp, nu]                      # [NS, 3, 3] shifted
            a = f.reshape(NFS, 128, 3, 3)
            dev = np.stack([a.real.transpose(1, 2, 3, 0),
                            a.imag.transpose(1, 2, 3, 0)], axis=3)
            out[mu * 4 + nu] = dev.reshape(128, 18 * NFS).astype(bf16)
    return out


def _pack_lhs(omega, bf16):
    """omega [8,10,4,3] complex -> lhsA [4,3,3,128,32], lhsB [4,3,2,64,32]."""
    lhsA = np.zeros((4, 3, 3, 128, 32), np.float32)
    lhsB = np.zeros((4, 3, 2, 64, 32), np.float32)
    for m in range(4):
        for k in range(3):
            om = omega[:, :, m, k]             # [i, j]
            for g in range(5):
                if g < 3:
                    tgt, rbase = lhsA[m, k, g], 0
                else:
                    tgt, rbase = lhsB[m, k, g - 3], 120
                for r in range(tgt.shape[0]):
                    ch = rbase + r
                    if ch >= 180:
                        continue
                    cd, jp = ch // 20, ch % 20
                    j, pw = jp // 2, jp % 2
                    if g < 4:
                        if cd not in (2 * g, 2 * g + 1):
                            continue
                        cdl = cd - 2 * g
                    else:
                        if cd != 8:
                            continue
                        cdl = 0
                    for i in range(8):
                        w = om[i, j]
                        col = cdl * 16 + i * 2
                        if pw == 0:
                            tgt[r, col] = w.real
                            tgt[r, col + 1] = w.imag
                        else:
                            tgt[r, col] = -w.imag
                            tgt[r, col + 1] = w.real
    lhsA2 = lhsA.transpose(3, 0, 1, 2, 4).reshape(128, 36 * 32)
    lhsB2 = lhsB.transpose(3, 0, 1, 2, 4).reshape(64, 24 * 32)
    return (np.ascontiguousarray(lhsA2).astype(bf16),
            np.ascontiguousarray(lhsB2).astype(bf16))


def _unpack_out(o):
    """o [128, 9, 144, 18] fp32 -> [NS, 8, 3, 3] complex64."""
    a = o.reshape(128, NCHUNK, 8, 3, 3, 2, CHUNK_FS)
    a = a.transpose(1, 6, 0, 2, 3, 4, 5)       # c fs p i a b rho
    a = a.reshape(NS, 8, 3, 3, 2)
    return (a[..., 0] + 1j * a[..., 1]).astype(np.complex64)


def _get_compiled():
    global _COMPILED
    if _COMPILED is None:
        import concourse.bacc as bacc
        nc = bacc.Bacc("TRN2", target_bir_lowering=False, debug=False,
                       num_devices=8)
        _build(nc)
        nc.compile()
        _COMPILED = nc
    return _COMPILED


def kernel(U, omega, K=1, N_out=8, **_):
    import ml_dtypes
    from concourse import bass_utils
    bf16 = ml_dtypes.bfloat16
    U = np.asarray(U)
    omega = np.asarray(omega)
    B = U.shape[0]
    lhsA, lhsB = _pack_lhs(omega, bf16)
    in_maps = []
    for b in range(B):
        Ub = U[b].reshape(NS, 4, 3, 3)
        in_maps.append({
            "u": _pack_u(Ub, bf16),
            "ush": _pack_ush(Ub, bf16),
            "lhsA": lhsA, "lhsB": lhsB,
        })
    nc = _get_compiled()
    res = bass_utils.run_bass_kernel_spmd(nc, in_maps, core_ids=list(range(8)))
    outs = []
    for b in range(B):
        outs.append(_unpack_out(res.results[b]["o"]).reshape(
            L, L, L, L, 8, 3, 3))
    return np.stack(outs).astype(np.complex64)



# revision 13
# speedup vs baseline: 3339.3009x; 3339.3009x over previous
"""Lattice gauge CNN (L-CNN) layer on 8 TRN2 NeuronCores via Bass.

Self-contained: host packing + device kernel + unpacking. Data-parallel over
batch (B=8 -> one full 12^4 lattice per core).

Device model (per core):
  flat site s = ((a0*12+a1)*12+a2)*12+a3;  p = s % 128;  fs = s // 128 (0..161)
  site-major tiles [128, nch*162] bf16, f = ch*162 + fs  (ch-outer, fs-inner)
  U channels: ch = mu*18 + (3a+c)*2 + rho.

Polyakov-loop shifts go through a duplicated DRAM staging buffer: a global
flat-index shift becomes one linear read, and the per-axis wrap correction is a
second linear read blended in with a precomputed mask (copy_predicated).
W transposition to channel-major uses multi-block DMA transposes.
"""
import numpy as np

L = 12
NS = L ** 4            # 20736
NFS = NS // 128        # 162
CHUNK_FS = 18          # 9 chunks x 18 fs
NCHUNK = NFS // CHUNK_FS
CHS = CHUNK_FS * 128   # sites per chunk = 2304
WIN = 384              # MM window; 2304/384 = 6 windows per chunk
NWIN = CHS // WIN
NCH_U = 72
SIG = (1728, 144, 12, 1)
WSLOT = 192            # per-fs pitch in w_t: A block slots 0..127, B 64..191
ABASE = (0, 60, 128)   # w_t slot base per color-row a (cd = 3a + b, stride 20)

_COMPILED = None
_RUNNER = None
DEBUG_WCM = False


# --------------------------------------------------------------------------
# compile-time site maps
# --------------------------------------------------------------------------
def _site_shift_map(mu, delta):
    idx = np.arange(NS).reshape(L, L, L, L)
    return np.roll(idx, -delta, axis=mu).reshape(-1)


def _col_groups(src_map, w0, n):
    """Decompose shifted-window read into (dst0, src0, step, nblk, ln)."""
    src = src_map[w0:w0 + n]
    runs, st = [], 0
    for i in range(1, n + 1):
        if i == n or src[i] != src[i - 1] + 1:
            runs.append((st, int(src[st]), i - st))
            st = i
    from collections import defaultdict
    byln = defaultdict(list)
    for (d0, s0, ln) in runs:
        byln[ln].append((d0, s0))
    groups = []
    for ln, items in byln.items():
        items.sort()
        i = 0
        while i < len(items):
            d0, s0 = items[i]
            j, step = i + 1, 0
            while j < len(items):
                dd, ss = items[j]
                st2 = dd - items[j - 1][0]
                if ss - items[j - 1][1] != st2:
                    break
                if j == i + 1:
                    step = st2
                elif st2 != step:
                    break
                j += 1
            groups.append((d0, s0, step if j - i > 1 else 0, j - i, ln))
            i = j
    return groups


# --------------------------------------------------------------------------
# device build
# --------------------------------------------------------------------------
def _build(nc):
    import concourse.mybir as mybir
    from concourse.tile import TileContext, add_dep_helper

    F32 = mybir.dt.float32
    BF16 = mybir.dt.bfloat16
    MUL = mybir.AluOpType.mult
    ADD = mybir.AluOpType.add
    SUB = mybir.AluOpType.subtract

    u_d = nc.dram_tensor("u", [128, NCH_U, NFS], BF16, kind="ExternalInput")
    ush_d = nc.dram_tensor("ush", [16, 128, 18 * NFS], BF16,
                           kind="ExternalInput")
    lhsA_d = nc.dram_tensor("lhsA", [128, 36 * 32], BF16, kind="ExternalInput")
    lhsB_d = nc.dram_tensor("lhsB", [128, 24 * 32], BF16, kind="ExternalInput")
    lhsM_d = nc.dram_tensor("lhsM", [128, 5 * 128], BF16, kind="ExternalInput")
    msk_d = nc.dram_tensor("msk", [128, 12 * NFS], mybir.dt.uint8,
                           kind="ExternalInput")
    o_d = nc.dram_tensor("o", [128, NCHUNK, 144, CHUNK_FS], F32,
                         kind="ExternalOutput")
    dbg_d = None
    dbgw_d = None
    if DEBUG_WCM:
        dbg_d = nc.dram_tensor("dbg", [2, 128, NS], BF16,
                               kind="ExternalOutput")
        dbgw_d = nc.dram_tensor("dbgw", [128, NFS * WSLOT], BF16,
                                kind="ExternalOutput")

    # Polyakov roll staging: [mu, level(V2/V4), ch, p, 3*NFS] — p-major with
    # the fs axis triplicated, so any flat shift becomes a partition-rotated
    # read (2 pieces) with an fs offset into the middle copy, wrap-free.
    roll_d = nc.dram_tensor("rolld", [4, 2, 18, 128, 3 * NFS], BF16,
                            kind="Internal")
    wdA_d = nc.dram_tensor("wdA", [128, NS], BF16, kind="Internal")
    wdB_d = nc.dram_tensor("wdB", [128, NS], BF16, kind="Internal")

    def APC(base, off, dims):
        t = type(base)
        ap = [list(base.ap[0])] + [[int(s), int(n)] for s, n in dims]
        return t(base.tensor, base.offset + int(off), ap)

    def DAP(base, off, dims):
        t = type(base)
        ap = [[int(s), int(n)] for s, n in dims]
        return t(base.tensor, base.offset + int(off), ap)

    def TT(out, a, b, op):
        return nc.vector.tensor_tensor(out=out, in0=a, in1=b, op=op)

    shift_maps = {(m, k): _site_shift_map(m, k) for m in range(4)
                  for k in (-1, 1)}

    with TileContext(nc) as tc:
        with tc.tile_pool(name="const", bufs=1) as cpool, \
             tc.tile_pool(name="sc", bufs=2) as scpool, \
             tc.tile_pool(name="a1p", bufs=3) as a1pool, \
             tc.tile_pool(name="ps", bufs=6, space="PSUM") as pspool:

            u_t = cpool.tile([128, NCH_U * NFS], BF16, tag="u")
            nc.sync.dma_start(
                u_t[:].rearrange("p (c f) -> p c f", c=NCH_U), u_d[:])
            lhsA_t = cpool.tile([128, 36 * 32], BF16, tag="lhsA")
            nc.scalar.dma_start(lhsA_t[:], lhsA_d[:])
            lhsB_t = cpool.tile([128, 24 * 32], BF16, tag="lhsB")
            nc.sync.dma_start(lhsB_t[:], lhsB_d[:])
            lhsM_t = cpool.tile([128, 5 * 128], BF16, tag="lhsM")
            nc.scalar.dma_start(lhsM_t[:], lhsM_d[:])

            # complex 3x3 matmul C = A @ B(^dag), full lattice
            def cmat(At, Ach0, Bt, Bch0, writer, adjB=False, pre_deps=None):
                e = NFS
                insts = []
                for a in range(3):
                    P = [scpool.tile([128, 9 * e], BF16, tag=f"pp{t}",
                                     name=f"pp{t}")
                         for t in range(4)]
                    for t, (ra, rb) in enumerate(
                            ((0, 0), (1, 1), (0, 1), (1, 0))):
                        a_ap = APC(At[:], (Ach0 + (3 * a) * 2 + ra) * NFS,
                                   [[0, 3], [2 * NFS, 3], [1, e]])
                        if not adjB:
                            bdims = [[2 * NFS, 3], [6 * NFS, 3], [1, e]]
                        else:
                            bdims = [[6 * NFS, 3], [2 * NFS, 3], [1, e]]
                        b_ap = APC(Bt[:], (Bch0 + rb) * NFS, bdims)
                        o_ap = APC(P[t][:], 0, [[3 * e, 3], [e, 3], [1, e]])
                        ii = TT(o_ap, a_ap, b_ap, MUL)
                        if not insts:
                            for d in (pre_deps or []):
                                add_dep_helper(ii.ins, d.ins,
                                               reason="cmat RAW")
                        insts.append(ii)
                    f = [APC(P[t][:], 0, [[1, 9 * e]]) for t in range(4)]
                    if not adjB:
                        TT(f[0], f[0], f[1], SUB)
                        TT(f[2], f[2], f[3], ADD)
                        Dr, Di = P[0], P[2]
                    else:
                        TT(f[0], f[0], f[1], ADD)
                        TT(f[3], f[3], f[2], SUB)
                        Dr, Di = P[0], P[3]
                    for rho, D in ((0, Dr), (1, Di)):
                        A1 = a1pool.tile([128, 3 * e], BF16, tag="a1",
                                         name="a1")
                        TT(APC(A1[:], 0, [[e, 3], [1, e]]),
                           APC(D[:], 0, [[3 * e, 3], [1, e]]),
                           APC(D[:], e, [[3 * e, 3], [1, e]]), ADD)
                        insts.append(writer(a, rho,
                               APC(A1[:], 0, [[e, 3], [1, e]]),
                               APC(D[:], 2 * e, [[3 * e, 3], [1, e]])))
                return insts[-1]

            def wr_link(tile):
                def w(a, rho, x_ap, y_ap):
                    o = APC(tile[:], ((3 * a) * 2 + rho) * NFS,
                            [[2 * NFS, 3], [1, NFS]])
                    return TT(o, x_ap, y_ap, ADD)
                return w

            # ------------- phase B: W production ------------------------
            with tc.tile_pool(name="wp", bufs=1) as wpool, \
                 tc.tile_pool(name="ush", bufs=3) as ushpool, \
                 tc.tile_pool(name="rp", bufs=3) as rpool, \
                 tc.tile_pool(name="vp", bufs=6) as vpool:
                w_t = wpool.tile([128, NFS * WSLOT], BF16, tag="w")
                nc.vector.memset(w_t[:], 0.0)
                msk_t = wpool.tile([128, 12 * NFS], mybir.dt.uint8, tag="msk")
                nc.scalar.dma_start(msk_t[:], msk_d[:])

                def wr_W(jw):
                    def w(a, rho, x_ap, y_ap):
                        o = APC(w_t[:], ABASE[a] + jw * 2 + rho,
                                [[20, 3], [WSLOT, NFS]])
                        return TT(o, x_ap, y_ap, ADD)
                    return w

                def vt():
                    return vpool.tile([128, 18 * NFS], BF16, tag="vt",
                                      name="vt")

                # plaquette emission is interleaved between Polyakov steps
                # so the roll DMAs hide under plaquette vector work
                pairs = [(mu, nu) for mu in range(4)
                         for nu in range(mu + 1, 4)]
                plaq_state = {"i": 0}

                def do_plaq(n):
                    for _ in range(n):
                        i = plaq_state["i"]
                        if i >= 6:
                            return
                        plaq_state["i"] = i + 1
                        mu, nu = pairs[i]
                        eng = nc.sync if i % 2 == 0 else nc.scalar
                        u1 = ushpool.tile([128, 18 * NFS], BF16, tag="ush",
                                          name="ush1")
                        eng.dma_start(u1[:], ush_d[mu * 4 + nu])
                        u2 = ushpool.tile([128, 18 * NFS], BF16, tag="ush",
                                          name="ush2")
                        eng.dma_start(u2[:], ush_d[nu * 4 + mu])
                        Rmn = rpool.tile([128, 18 * NFS], BF16, tag="rt")
                        cmat(u_t, mu * 18, u1, 0, wr_link(Rmn))
                        Rnm = rpool.tile([128, 18 * NFS], BF16, tag="rt")
                        cmat(u_t, nu * 18, u2, 0, wr_link(Rnm))
                        cmat(Rmn, 0, Rnm, 0, wr_W(i), adjB=True)

                # ---- Polyakov loops: log-doubling with DRAM-staged rolls
                FSX = 3 * NFS                  # triplicated fs extent
                CHP = 128 * FSX                # per-channel dram pitch

                def stage_roll(t, mu, lvl, deps):
                    base = (mu * 2 + lvl) * 18 * CHP
                    insts = []
                    for c in range(3):
                        eng = (nc.sync, nc.scalar, nc.sync)[c]
                        dst = DAP(roll_d[:], base + c * NFS,
                                  [[FSX, 128], [CHP, 18], [1, NFS]])
                        src = APC(t[:], 0, [[NFS, 18], [1, NFS]])
                        i = eng.dma_start(dst, src)
                        for d in deps:
                            add_dep_helper(i.ins, d.ins, reason="roll stage")
                        insts.append(i)
                    return insts

                DI = {2: 0, 4: 1, 8: 2}

                def read_roll(mu, lvl, delta, wdeps):
                    """dst[s'] = staged[(s'+D) mod NS] as a partition-rotated
                    read: two pieces split at p' = 128-rd, fs offset into the
                    middle fs copy."""
                    base = (mu * 2 + lvl) * 18 * CHP
                    sig = SIG[mu]
                    va = vt()
                    vb = vt()
                    rds = []
                    for t, D in ((va, delta * sig),
                                 (vb, delta * sig - 12 * sig)):
                        qd, rd = D // 128, D % 128
                        pieces = [(0, 128 - rd, rd, NFS + qd)]
                        if rd:
                            pieces.append((128 - rd, rd, 0, NFS + qd + 1))
                        for pi_, (p0, pn, sp, f0) in enumerate(pieces):
                            eng = nc.sync if (pi_ + (t is vb)) % 2 == 0 \
                                else nc.scalar
                            i = eng.dma_start(
                                APC(t[p0:p0 + pn, :], 0,
                                    [[NFS, 18], [1, NFS]]),
                                DAP(roll_d[:], base + sp * FSX + f0,
                                    [[FSX, pn], [CHP, 18], [1, NFS]]))
                            rds.append(i)
                    for i in rds:
                        for d in wdeps:
                            add_dep_helper(i.ins, d.ins, reason="roll RAW")
                    return va, vb, tuple(rds)

                def blend(mu, delta, va, vb, rdeps):
                    mask = APC(msk_t[:], (mu * 3 + DI[delta]) * NFS,
                               [[0, 18], [1, NFS]])
                    pi = nc.vector.copy_predicated(
                        APC(va[:], 0, [[NFS, 18], [1, NFS]]), mask,
                        APC(vb[:], 0, [[NFS, 18], [1, NFS]]))
                    for d in rdeps:
                        add_dep_helper(pi.ins, d.ins, reason="blend RAW")
                    return pi

                for mu in range(4):
                    uv = ushpool.tile([128, 18 * NFS], BF16, tag="ush",
                                      name="ushv")
                    (nc.sync if mu % 2 else nc.scalar).dma_start(
                        uv[:], ush_d[mu * 4 + mu])
                    V2 = vt()
                    v2w = cmat(u_t, mu * 18, uv, 0, wr_link(V2))
                    sd = stage_roll(V2, mu, 0, [v2w])
                    va2, vb2, rd2 = read_roll(mu, 0, 2, sd)
                    do_plaq(1)
                    pi = blend(mu, 2, va2, vb2, rd2)
                    V4 = vt()
                    v4w = cmat(V2, 0, va2, 0, wr_link(V4), pre_deps=[pi])
                    sd = stage_roll(V4, mu, 1, [v4w])
                    va4, vb4, rd4 = read_roll(mu, 1, 4, sd)
                    va8, vb8, rd8 = read_roll(mu, 1, 8, sd)
                    do_plaq(1)
                    pi4 = blend(mu, 4, va4, vb4, rd4)
                    V8 = vt()
                    cmat(V4, 0, va4, 0, wr_link(V8), pre_deps=[pi4])
                    pi8 = blend(mu, 8, va8, vb8, rd8)
                    cmat(V8, 0, va8, 0, wr_W(6 + mu), pre_deps=[pi8])
                do_plaq(6)

                if DEBUG_WCM:
                    nc.sync.dma_start(dbgw_d[:], w_t[:])
                # ---- W -> DRAM (pitch-128 A/B blocks) ----
                w_v = w_t[:].rearrange("p (f s) -> p f s", s=WSLOT)
                iA = nc.scalar.dma_start(
                    wdA_d[:].rearrange("p (f c) -> p f c", c=128),
                    w_v[:, :, 0:128])
                iB = nc.scalar.dma_start(
                    wdB_d[:].rearrange("p (f c) -> p f c", c=128),
                    w_v[:, :, 64:192])

            # ---------- phase C: W -> channel-major (multi-block xbar) ---
            with tc.tile_pool(name="wcmp", bufs=1) as wcmpool, \
                 tc.tile_pool(name="chk", bufs=1) as chkpool, \
                 tc.tile_pool(name="tcm", bufs=3) as tcmpool:
                wcmA = wcmpool.tile([128, NS], BF16, tag="wcmA")
                wcmB = wcmpool.tile([128, NS], BF16, tag="wcmB")
                # ALL transposes live on the sync ring only (serialized —
                # concurrent xbar transposes on both rings corrupt data);
                # producers sit on the scalar ring so RAW deps are
                # cross-queue semaphores.
                tA = nc.sync.dma_start_transpose(
                    wcmA[:].rearrange("p (f c) -> p f c", c=128), wdA_d[:])
                add_dep_helper(tA.ins, iA.ins, reason="w xpose RAW")
                tB = nc.sync.dma_start_transpose(
                    wcmB[:].rearrange("p (f c) -> p f c", c=128), wdB_d[:])
                add_dep_helper(tB.ins, iB.ins, reason="w xpose RAW")
                if DEBUG_WCM:
                    nc.scalar.dma_start(dbg_d[0], wcmA[:])
                    nc.scalar.dma_start(dbg_d[1], wcmB[:])

                # ---------- phase D: per-chunk T + sandwich -------------
                traw = chkpool.tile([128, CHUNK_FS * 640], BF16, tag="traw")
                t2_t = chkpool.tile([128, CHUNK_FS * 576], BF16, tag="t2")
                x_t = chkpool.tile([128, CHUNK_FS * 144], BF16, tag="x")
                oc_t = chkpool.tile([128, CHUNK_FS * 144], F32, tag="oc")

                e = CHUNK_FS
                for c in range(NCHUNK):
                    fs0 = c * CHUNK_FS
                    for wi in range(NWIN):
                        w0 = c * CHS + wi * WIN
                        for g in range(5):
                            ps = pspool.tile([128, WIN], F32, tag="psg",
                                             name="psg")
                            wcm = wcmA if g < 3 else wcmB
                            # k=0 for all 4 mu in one 128-wide matmul
                            nc.tensor.matmul(
                                ps[:, :], lhsM_t[:, g * 128:(g + 1) * 128],
                                wcm[:, w0:w0 + WIN],
                                start=True, stop=False, tile_position=(0, 0))
                            for m in range(4):
                                M = 32 if g < 4 else 16
                                if g < 3:
                                    def lh(k):
                                        i0 = ((m * 3 + k) * 3 + g) * 32
                                        return lhsA_t[:, i0:i0 + 32]
                                else:
                                    def lh(k):
                                        i0 = ((m * 3 + k) * 2 + (g - 3)) * 32
                                        return lhsB_t[:, i0:i0 + M]
                                po = ps[32 * m:32 * m + M, :]
                                pieces = []
                                for k, kk in ((-1, 0), (1, 2)):
                                    for grp in _col_groups(
                                            shift_maps[(m, k)], w0, WIN):
                                        pieces.append((kk,) + grp)
                                last = (m == 3)
                                for idx, (kk, d0, s0, stp, nb, ln) in \
                                        enumerate(pieces):
                                    cdims = ([[stp, nb], [1, ln]] if nb > 1
                                             else [[1, ln]])
                                    o_ap = APC(po, d0, cdims)
                                    r_ap = APC(wcm[:, :], s0, cdims)
                                    nc.tensor.matmul(
                                        o_ap, lh(kk), r_ap, start=False,
                                        stop=(last and
                                              idx == len(pieces) - 1),
                                        tile_position=(0, 32 * m))
                            tcm = tcmpool.tile([128, WIN], BF16,
                                               tag=f"tcm{g}", name=f"tcm{g}")
                            nc.scalar.copy(tcm[:], ps[:])
                            nc.sync.dma_start_transpose(
                                APC(traw[:], (wi * 3) * 640 + g * 128,
                                    [[640, 3], [1, 128]]),
                                tcm[:])

                    # repack traw -> t2 (ch-outer, clean cd strides), on ACT
                    for m in range(4):
                        for par in (0, 1):
                            ng = 5 - par
                            for pr in (0, 1):
                                i_ap = APC(traw[:], m * 32 + par * 16 + pr,
                                           [[2, 8], [128, ng], [640, e]])
                                o_ap = APC(t2_t[:], (m * 144 + par * 2 + pr) * e,
                                           [[18 * e, 8], [4 * e, ng], [1, e]])
                                nc.scalar.copy(o_ap, i_ap)

                    # sandwich: out = sum_m U_m @ T_m @ U_m^dag
                    for m in range(4):
                        # stage 1: X[i,a,b] = sum_c U[a,c] T[i,3c+b]
                        for a in range(3):
                            for b in range(3):
                                P = [scpool.tile([128, 24 * e], BF16,
                                                 tag=f"pp{t}", name=f"sp{t}")
                                     for t in range(4)]
                                for t, (ru, rt) in enumerate(
                                        ((0, 0), (1, 1), (0, 1), (1, 0))):
                                    uap = APC(u_t[:],
                                              (m * 18 + (3 * a) * 2 + ru) * NFS + fs0,
                                              [[0, 8], [2 * NFS, 3], [1, e]])
                                    tap = APC(t2_t[:], (m * 144 + b * 2 + rt) * e,
                                              [[18 * e, 8], [6 * e, 3], [1, e]])
                                    o_ap = APC(P[t][:], 0,
                                               [[3 * e, 8], [e, 3], [1, e]])
                                    TT(o_ap, uap, tap, MUL)
                                f = [APC(P[t][:], 0, [[1, 24 * e]]) for t in range(4)]
                                TT(f[0], f[0], f[1], SUB)
                                TT(f[2], f[2], f[3], ADD)
                                for rho, D in ((0, P[0]), (1, P[2])):
                                    A1 = a1pool.tile([128, 8 * e], BF16, tag="sa1", name="sa1")
                                    TT(APC(A1[:], 0, [[e, 8], [1, e]]),
                                       APC(D[:], 0, [[3 * e, 8], [1, e]]),
                                       APC(D[:], e, [[3 * e, 8], [1, e]]), ADD)
                                    xo = APC(x_t[:], ((a * 3 + b) * 2 + rho) * e,
                                             [[18 * e, 8], [1, e]])
                                    TT(xo, APC(A1[:], 0, [[e, 8], [1, e]]),
                                       APC(D[:], 2 * e, [[3 * e, 8], [1, e]]), ADD)
                        # stage 2: out[i,a,b] += sum_d X[i,a,d] conj(U[b,d])
                        for a in range(3):
                            for b in range(3):
                                P = [scpool.tile([128, 24 * e], BF16,
                                                 tag=f"pp{t}", name=f"sp{t}")
                                     for t in range(4)]
                                for t, (rx, ru) in enumerate(
                                        ((0, 0), (1, 1), (0, 1), (1, 0))):
                                    xap = APC(x_t[:], ((a * 3) * 2 + rx) * e,
                                              [[18 * e, 8], [2 * e, 3], [1, e]])
                                    uap = APC(u_t[:],
                                              (m * 18 + (3 * b) * 2 + ru) * NFS + fs0,
                                              [[0, 8], [2 * NFS, 3], [1, e]])
                                    o_ap = APC(P[t][:], 0,
                                               [[3 * e, 8], [e, 3], [1, e]])
                                    TT(o_ap, xap, uap, MUL)
                                f = [APC(P[t][:], 0, [[1, 24 * e]]) for t in range(4)]
                                TT(f[0], f[0], f[1], ADD)      # XrUr + XiUi
                                TT(f[3], f[3], f[2], SUB)      # XiUr - XrUi
                                for rho, D in ((0, P[0]), (1, P[3])):
                                    A1 = a1pool.tile([128, 8 * e], BF16, tag="sa1", name="sa1")
                                    TT(APC(A1[:], 0, [[e, 8], [1, e]]),
                                       APC(D[:], 0, [[3 * e, 8], [1, e]]),
                                       APC(D[:], e, [[3 * e, 8], [1, e]]), ADD)
                                    oo = APC(oc_t[:], ((a * 3 + b) * 2 + rho) * e,
                                             [[18 * e, 8], [1, e]])
                                    if m == 0:
                                        TT(oo, APC(A1[:], 0, [[e, 8], [1, e]]),
                                           APC(D[:], 2 * e, [[3 * e, 8], [1, e]]),
                                           ADD)
                                    else:
                                        A2 = a1pool.tile([128, 8 * e], BF16,
                                                         tag="sa2", name="sa2")
                                        TT(APC(A2[:], 0, [[e, 8], [1, e]]),
                                           APC(A1[:], 0, [[e, 8], [1, e]]),
                                           APC(D[:], 2 * e, [[3 * e, 8], [1, e]]),
                                           ADD)
                                        TT(oo, oo, APC(A2[:], 0, [[e, 8], [1, e]]),
                                           ADD)
                    (nc.sync if c % 2 == 0 else nc.scalar).dma_start(
                        o_d[:, c], oc_t[:].rearrange(
                            "p (ch f) -> p ch f", ch=144))
    return nc


# --------------------------------------------------------------------------
# host side
# --------------------------------------------------------------------------
def _pack_u(Ub, bf16):
    """Ub [NS, 4, 3, 3] complex64 -> [128, 72, 162] bf16."""
    arr = Ub.reshape(NFS, 128, 4, 3, 3)
    re = arr.real.transpose(1, 2, 3, 4, 0)     # p mu a c fs
    im = arr.imag.transpose(1, 2, 3, 4, 0)
    dev = np.stack([re, im], axis=4)           # p mu a c rho fs
    return np.ascontiguousarray(dev.reshape(128, NCH_U, NFS)).astype(bf16)


def _pack_ush(Ub, bf16):
    """16 host-shifted link fields -> [16, 128, 18*162] bf16."""
    out = np.empty((16, 128, 18 * NFS), bf16)
    for mu in range(4):
        mp = _site_shift_map(mu, 1)
        for nu in range(4):
            f = Ub[mp, nu]                      # [NS, 3, 3] shifted
            a = f.reshape(NFS, 128, 3, 3)
            dev = np.stack([a.real.transpose(1, 2, 3, 0),
                            a.imag.transpose(1, 2, 3, 0)], axis=3)
            out[mu * 4 + nu] = dev.reshape(128, 18 * NFS).astype(bf16)
    return out


def _pack_msk(bf16):
    """Wrap masks for the Polyakov rolls: [128, 12*162] (mu*3+di major)."""
    s = np.arange(NS)
    out = np.zeros((12, NS), np.float32)
    for mu in range(4):
        ax = (s // SIG[mu]) % 12
        for di, d in enumerate((2, 4, 8)):
            out[mu * 3 + di] = (ax >= 12 - d).astype(np.float32)
    o = out.reshape(12, NFS, 128).transpose(2, 0, 1)
    return np.ascontiguousarray(o.reshape(128, 12 * NFS)).astype(np.uint8)


def _pack_lhs(omega, bf16):
    """omega [8,10,4,3] complex -> lhsA [128,36*32], lhsB [128,24*32],
    lhsM [128,5*128] (merged k=0 weights, 4 mu side by side)."""
    rowsA = [(r, r) for r in range(120)]
    rowsB = [(r, 56 + r) for r in range(64, 124)]

    def fill(tgt, rows_ch, m, k, g, colbase=0):
        om = omega[:, :, m, k]
        ncd = 2 if g < 4 else 1
        cd0 = 2 * g if g < 4 else 8
        for r, ch in rows_ch:
            cd, jp = ch // 20, ch % 20
            j, pw = jp // 2, jp % 2
            if not (cd0 <= cd < cd0 + ncd):
                continue
            cdl = cd - cd0
            w = om[:, j]
            for i in range(8):
                col = colbase + cdl * 16 + i * 2
                if pw == 0:
                    tgt[r, col] = w[i].real
                    tgt[r, col + 1] = w[i].imag
                else:
                    tgt[r, col] = -w[i].imag
                    tgt[r, col + 1] = w[i].real

    lhsA = np.zeros((4, 3, 3, 128, 32), np.float32)
    lhsB = np.zeros((4, 3, 2, 128, 32), np.float32)
    for m in range(4):
        for k in range(3):
            for g in range(5):
                if g < 3:
                    fill(lhsA[m, k, g], rowsA, m, k, g)
                else:
                    fill(lhsB[m, k, g - 3], rowsB, m, k, g)
    lhsM = np.zeros((128, 5 * 128), np.float32)
    for g in range(5):
        for m in range(4):
            fill(lhsM, rowsA if g < 3 else rowsB, m, 1, g,
                 colbase=g * 128 + m * 32)
    lhsA2 = lhsA.transpose(3, 0, 1, 2, 4).reshape(128, 36 * 32)
    lhsB2 = lhsB.transpose(3, 0, 1, 2, 4).reshape(128, 24 * 32)
    return (np.ascontiguousarray(lhsA2).astype(bf16),
            np.ascontiguousarray(lhsB2).astype(bf16),
            np.ascontiguousarray(lhsM).astype(bf16))


def _unpack_out(o):
    """o [128, 9, 144, 18] fp32 -> [NS, 8, 3, 3] complex64."""
    a = o.reshape(128, NCHUNK, 8, 3, 3, 2, CHUNK_FS)
    a = a.transpose(1, 6, 0, 2, 3, 4, 5)       # c fs p i a b rho
    a = a.reshape(NS, 8, 3, 3, 2)
    return (a[..., 0] + 1j * a[..., 1]).astype(np.complex64)


def _get_compiled():
    global _COMPILED
    if _COMPILED is None:
        import concourse.bacc as bacc
        nc = bacc.Bacc("TRN2", target_bir_lowering=False, debug=False,
                       num_devices=8)
        _build(nc)
        nc.compile()
        _COMPILED = nc
    return _COMPILED


def _run(nc, in_maps):
    """Cached-jit SPMD dispatch (the stock path re-jits on every call)."""
    global _RUNNER
    import jax
    import numpy as np
    from jax.sharding import Mesh, PartitionSpec
    from jax.experimental.shard_map import shard_map
    from concourse import bass2jax, mybir

    n = len(in_maps)
    if _RUNNER is None:
        bass2jax.install_neuronx_cc_hook()
        partition_name = (nc.partition_id_tensor.name
                          if nc.partition_id_tensor else None)
        in_names, out_names, out_avals = [], [], []
        for alloc in nc.m.functions[0].allocations:
            if not isinstance(alloc, mybir.MemoryLocationSet):
                continue
            name = alloc.memorylocations[0].name
            if alloc.kind == "ExternalInput":
                if name != partition_name:
                    in_names.append(name)
            elif alloc.kind == "ExternalOutput":
                out_names.append(name)
                out_avals.append(jax.core.ShapedArray(
                    tuple(alloc.tensor_shape), mybir.dt.np(alloc.dtype)))
        n_params = len(in_names)
        all_names = list(in_names) + list(out_names)
        if partition_name is not None:
            all_names.append(partition_name)
        donate = tuple(range(n_params, n_params + len(out_names)))

        def _body(*args):
            operands = list(args)
            if partition_name is not None:
                operands.append(bass2jax.partition_id_tensor())
            outs = bass2jax._bass_exec_p.bind(
                *operands,
                out_avals=tuple(out_avals),
                in_names=tuple(all_names),
                out_names=tuple(out_names),
                lowering_input_output_aliases=(),
                sim_require_finite=True,
                sim_require_nnan=True,
                nc=nc,
            )
            return tuple(outs)

        devices = jax.devices()[:n]
        mesh = Mesh(np.asarray(devices), ("core",))
        specs = (PartitionSpec("core"),) * (n_params + len(out_names))
        jitted = jax.jit(
            shard_map(_body, mesh=mesh, in_specs=specs,
                      out_specs=(PartitionSpec("core"),) * len(out_names),
                      check_rep=False),
            donate_argnums=donate, keep_unused=True)
        _RUNNER = (jitted, in_names, out_names, out_avals)

    jitted, in_names, out_names, out_avals = _RUNNER
    concat_in = [np.concatenate([m[name] for m in in_maps], axis=0)
                 for name in in_names]
    concat_zeros = [np.zeros((n * a.shape[0],) + tuple(a.shape[1:]), a.dtype)
                    for a in out_avals]
    out_arrs = jitted(*concat_in, *concat_zeros)
    return [
        {name: np.asarray(out_arrs[i]).reshape(
            (n,) + tuple(out_avals[i].shape))[c]
         for i, name in enumerate(out_names)}
        for c in range(n)
    ]


def kernel(U, omega, K=1, N_out=8, **_):
    import ml_dtypes
    bf16 = ml_dtypes.bfloat16
    U = np.asarray(U)
    omega = np.asarray(omega)
    B = U.shape[0]
    lhsA, lhsB, lhsM = _pack_lhs(omega, bf16)
    msk = _pack_msk(bf16)
    in_maps = []
    for b in range(B):
        Ub = U[b].reshape(NS, 4, 3, 3)
        in_maps.append({
            "u": _pack_u(Ub, bf16),
            "ush": _pack_ush(Ub, bf16),
            "lhsA": lhsA, "lhsB": lhsB, "lhsM": lhsM, "msk": msk,
        })
    nc = _get_compiled()
    try:
        results = _run(nc, in_maps)
    except Exception:
        from concourse import bass_utils
        results = bass_utils.run_bass_kernel_spmd(
            nc, in_maps, core_ids=list(range(8))).results
    outs = []
    for b in range(B):
        outs.append(_unpack_out(results[b]["o"]).reshape(
            L, L, L, L, 8, 3, 3))
    return np.stack(outs).astype(np.complex64)


# revision 17
# speedup vs baseline: 3859.8455x; 1.1559x over previous
"""Lattice gauge CNN (L-CNN) layer on 8 TRN2 NeuronCores via Bass.

Self-contained: host packing + device kernel + unpacking. Data-parallel over
batch (B=8 -> one full 12^4 lattice per core).

Device model (per core):
  flat site s = ((a0*12+a1)*12+a2)*12+a3;  p = s % 128;  fs = s // 128 (0..161)
  site-major tiles [128, nch*162] bf16, f = ch*162 + fs  (ch-outer, fs-inner)
  U channels: ch = mu*18 + (3a+c)*2 + rho.

Polyakov-loop shifts go through a duplicated DRAM staging buffer: a global
flat-index shift becomes one linear read, and the per-axis wrap correction is a
second linear read blended in with a precomputed mask (copy_predicated).
W transposition to channel-major uses multi-block DMA transposes.
"""
import numpy as np

L = 12
NS = L ** 4            # 20736
NFS = NS // 128        # 162
CHUNK_FS = 18          # 9 chunks x 18 fs
NCHUNK = NFS // CHUNK_FS
CHS = CHUNK_FS * 128   # sites per chunk = 2304
WIN = 384              # MM window; 2304/384 = 6 windows per chunk
NWIN = CHS // WIN
NCH_U = 72
SIG = (1728, 144, 12, 1)
WSLOT = 192            # per-fs pitch in w_t: A block slots 0..127, B 64..191
ABASE = (0, 60, 128)   # w_t slot base per color-row a (cd = 3a + b, stride 20)

_COMPILED = None
_RUNNER = None
DEBUG_WCM = False


# --------------------------------------------------------------------------
# compile-time site maps
# --------------------------------------------------------------------------
def _site_shift_map(mu, delta):
    idx = np.arange(NS).reshape(L, L, L, L)
    return np.roll(idx, -delta, axis=mu).reshape(-1)


def _col_groups(src_map, w0, n):
    """Decompose shifted-window read into (dst0, src0, step, nblk, ln)."""
    src = src_map[w0:w0 + n]
    runs, st = [], 0
    for i in range(1, n + 1):
        if i == n or src[i] != src[i - 1] + 1:
            runs.append((st, int(src[st]), i - st))
            st = i
    from collections import defaultdict
    byln = defaultdict(list)
    for (d0, s0, ln) in runs:
        byln[ln].append((d0, s0))
    groups = []
    for ln, items in byln.items():
        items.sort()
        i = 0
        while i < len(items):
            d0, s0 = items[i]
            j, step = i + 1, 0
            while j < len(items):
                dd, ss = items[j]
                st2 = dd - items[j - 1][0]
                if ss - items[j - 1][1] != st2:
                    break
                if j == i + 1:
                    step = st2
                elif st2 != step:
                    break
                j += 1
            groups.append((d0, s0, step if j - i > 1 else 0, j - i, ln))
            i = j
    return groups


# --------------------------------------------------------------------------
# device build
# --------------------------------------------------------------------------
def _build(nc):
    import concourse.mybir as mybir
    from concourse.tile import TileContext, add_dep_helper

    F32 = mybir.dt.float32
    BF16 = mybir.dt.bfloat16
    MUL = mybir.AluOpType.mult
    ADD = mybir.AluOpType.add
    SUB = mybir.AluOpType.subtract

    u_d = nc.dram_tensor("u", [128, NCH_U, NFS], BF16, kind="ExternalInput")
    ush_d = nc.dram_tensor("ush", [16, 128, 18 * NFS], BF16,
                           kind="ExternalInput")
    lhsA_d = nc.dram_tensor("lhsA", [128, 36 * 32], BF16, kind="ExternalInput")
    lhsB_d = nc.dram_tensor("lhsB", [128, 24 * 32], BF16, kind="ExternalInput")
    lhsM_d = nc.dram_tensor("lhsM", [128, 5 * 128], BF16, kind="ExternalInput")
    msk_d = nc.dram_tensor("msk", [128, 12 * NFS], mybir.dt.uint8,
                           kind="ExternalInput")
    o_d = nc.dram_tensor("o", [128, NCHUNK, 144, CHUNK_FS], F32,
                         kind="ExternalOutput")
    dbg_d = None
    dbgw_d = None
    if DEBUG_WCM:
        dbg_d = nc.dram_tensor("dbg", [2, 128, NS], BF16,
                               kind="ExternalOutput")
        dbgw_d = nc.dram_tensor("dbgw", [128, NFS * WSLOT], BF16,
                                kind="ExternalOutput")

    # Polyakov roll staging: [mu, level(V2/V4), ch, p, 3*NFS] — p-major with
    # the fs axis triplicated, so any flat shift becomes a partition-rotated
    # read (2 pieces) with an fs offset into the middle copy, wrap-free.
    roll_d = nc.dram_tensor("rolld", [4, 2, 18, 128, 3 * NFS], BF16,
                            kind="Internal")
    wdA_d = nc.dram_tensor("wdA", [128, NS], BF16, kind="Internal")
    wdB_d = nc.dram_tensor("wdB", [128, NS], BF16, kind="Internal")

    def APC(base, off, dims):
        t = type(base)
        ap = [list(base.ap[0])] + [[int(s), int(n)] for s, n in dims]
        return t(base.tensor, base.offset + int(off), ap)

    def DAP(base, off, dims):
        t = type(base)
        ap = [[int(s), int(n)] for s, n in dims]
        return t(base.tensor, base.offset + int(off), ap)

    def TT(out, a, b, op):
        return nc.vector.tensor_tensor(out=out, in0=a, in1=b, op=op)

    shift_maps = {(m, k): _site_shift_map(m, k) for m in range(4)
                  for k in (-1, 1)}

    with TileContext(nc) as tc:
        with tc.tile_pool(name="const", bufs=1) as cpool, \
             tc.tile_pool(name="a1p", bufs=3) as a1pool, \
             tc.tile_pool(name="ps", bufs=6, space="PSUM") as pspool:

            u_t = cpool.tile([128, NCH_U * NFS], BF16, tag="u")
            nc.sync.dma_start(
                u_t[:].rearrange("p (c f) -> p c f", c=NCH_U), u_d[:])
            lhsA_t = cpool.tile([128, 36 * 32], BF16, tag="lhsA")
            nc.scalar.dma_start(lhsA_t[:], lhsA_d[:])
            lhsB_t = cpool.tile([128, 24 * 32], BF16, tag="lhsB")
            nc.sync.dma_start(lhsB_t[:], lhsB_d[:])
            lhsM_t = cpool.tile([128, 5 * 128], BF16, tag="lhsM")
            nc.scalar.dma_start(lhsM_t[:], lhsM_d[:])

            # complex 3x3 matmul C = A @ B(^dag), full lattice.
            # The 4 real-product quadrants live side by side in ONE tile so
            # both rho (Re/Im) planes fuse into single TT instructions.
            pb_cell = {}

            def cmat(At, Ach0, Bt, Bch0, writer, adjB=False, pre_deps=None):
                e = NFS
                insts = []
                for a in range(3):
                    P = pb_cell["pool"].tile([128, 4 * 9 * e], BF16,
                                             tag="pq", name="pq")
                    for t, (ra, rb) in enumerate(
                            ((0, 0), (1, 1), (0, 1), (1, 0))):
                        a_ap = APC(At[:], (Ach0 + (3 * a) * 2 + ra) * NFS,
                                   [[0, 3], [2 * NFS, 3], [1, e]])
                        if not adjB:
                            bdims = [[2 * NFS, 3], [6 * NFS, 3], [1, e]]
                        else:
                            bdims = [[6 * NFS, 3], [2 * NFS, 3], [1, e]]
                        b_ap = APC(Bt[:], (Bch0 + rb) * NFS, bdims)
                        o_ap = APC(P[:], t * 9 * e,
                                   [[3 * e, 3], [e, 3], [1, e]])
                        ii = TT(o_ap, a_ap, b_ap, MUL)
                        if not insts:
                            for d in (pre_deps or []):
                                add_dep_helper(ii.ins, d.ins,
                                               reason="cmat RAW")
                        insts.append(ii)
                    f = [APC(P[:], t * 9 * e, [[1, 9 * e]]) for t in range(4)]
                    if not adjB:
                        TT(f[0], f[0], f[1], SUB)
                        TT(f[2], f[2], f[3], ADD)
                        rq = 2 * 9 * e              # rho: q0 -> q2
                    else:
                        TT(f[0], f[0], f[1], ADD)
                        TT(f[3], f[3], f[2], SUB)
                        rq = 3 * 9 * e              # rho: q0 -> q3
                    A1 = a1pool.tile([128, 2 * 3 * e], BF16, tag="a1",
                                     name="a1")
                    TT(APC(A1[:], 0, [[3 * e, 2], [e, 3], [1, e]]),
                       APC(P[:], 0, [[rq, 2], [3 * e, 3], [1, e]]),
                       APC(P[:], e, [[rq, 2], [3 * e, 3], [1, e]]), ADD)
                    insts.append(writer(
                        a, APC(A1[:], 0, [[3 * e, 2], [e, 3], [1, e]]),
                        APC(P[:], 2 * e, [[rq, 2], [3 * e, 3], [1, e]])))
                return insts[-1]

            def wr_link(tile):
                def w(a, x_ap, y_ap):
                    o = APC(tile[:], (3 * a) * 2 * NFS,
                            [[NFS, 2], [2 * NFS, 3], [1, NFS]])
                    return TT(o, x_ap, y_ap, ADD)
                return w

            # ------------- phase B: W production ------------------------
            with tc.tile_pool(name="wp", bufs=1) as wpool, \
                 tc.tile_pool(name="ush", bufs=3) as ushpool, \
                 tc.tile_pool(name="rp", bufs=3) as rpool, \
                 tc.tile_pool(name="pb", bufs=1) as pbpool, \
                 tc.tile_pool(name="vp", bufs=6) as vpool:
                pb_cell["pool"] = pbpool
                w_t = wpool.tile([128, NFS * WSLOT], BF16, tag="w")
                nc.vector.memset(w_t[:], 0.0)
                msk_t = wpool.tile([128, 12 * NFS], mybir.dt.uint8, tag="msk")
                nc.scalar.dma_start(msk_t[:], msk_d[:])

                def wr_W(jw):
                    def w(a, x_ap, y_ap):
                        o = APC(w_t[:], ABASE[a] + jw * 2,
                                [[1, 2], [20, 3], [WSLOT, NFS]])
                        return TT(o, x_ap, y_ap, ADD)
                    return w

                def vt():
                    return vpool.tile([128, 18 * NFS], BF16, tag="vt",
                                      name="vt")

                # plaquette emission is interleaved between Polyakov steps
                # so the roll DMAs hide under plaquette vector work
                pairs = [(mu, nu) for mu in range(4)
                         for nu in range(mu + 1, 4)]
                plaq_state = {"i": 0}

                def do_plaq(n):
                    for _ in range(n):
                        i = plaq_state["i"]
                        if i >= 6:
                            return
                        plaq_state["i"] = i + 1
                        mu, nu = pairs[i]
                        eng = nc.sync if i % 2 == 0 else nc.scalar
                        u1 = ushpool.tile([128, 18 * NFS], BF16, tag="ush",
                                          name="ush1")
                        eng.dma_start(u1[:], ush_d[mu * 4 + nu])
                        u2 = ushpool.tile([128, 18 * NFS], BF16, tag="ush",
                                          name="ush2")
                        eng.dma_start(u2[:], ush_d[nu * 4 + mu])
                        Rmn = rpool.tile([128, 18 * NFS], BF16, tag="rt")
                        cmat(u_t, mu * 18, u1, 0, wr_link(Rmn))
                        Rnm = rpool.tile([128, 18 * NFS], BF16, tag="rt")
                        cmat(u_t, nu * 18, u2, 0, wr_link(Rnm))
                        cmat(Rmn, 0, Rnm, 0, wr_W(i), adjB=True)

                # ---- Polyakov loops: log-doubling with DRAM-staged rolls
                FSX = 3 * NFS                  # triplicated fs extent
                CHP = 128 * FSX                # per-channel dram pitch

                def stage_roll(t, mu, lvl, deps):
                    base = (mu * 2 + lvl) * 18 * CHP
                    insts = []
                    for c in range(3):
                        eng = (nc.sync, nc.scalar, nc.sync)[c]
                        dst = DAP(roll_d[:], base + c * NFS,
                                  [[FSX, 128], [CHP, 18], [1, NFS]])
                        src = APC(t[:], 0, [[NFS, 18], [1, NFS]])
                        i = eng.dma_start(dst, src)
                        for d in deps:
                            add_dep_helper(i.ins, d.ins, reason="roll stage")
                        insts.append(i)
                    return insts

                DI = {2: 0, 4: 1, 8: 2}

                def read_roll(mu, lvl, delta, wdeps):
                    """dst[s'] = staged[(s'+D) mod NS] as a partition-rotated
                    read: two pieces split at p' = 128-rd, fs offset into the
                    middle fs copy."""
                    base = (mu * 2 + lvl) * 18 * CHP
                    sig = SIG[mu]
                    va = vt()
                    vb = vt()
                    rds = []
                    for t, D in ((va, delta * sig),
                                 (vb, delta * sig - 12 * sig)):
                        qd, rd = D // 128, D % 128
                        pieces = [(0, 128 - rd, rd, NFS + qd)]
                        if rd:
                            pieces.append((128 - rd, rd, 0, NFS + qd + 1))
                        for pi_, (p0, pn, sp, f0) in enumerate(pieces):
                            eng = nc.sync if (pi_ + (t is vb)) % 2 == 0 \
                                else nc.scalar
                            i = eng.dma_start(
                                APC(t[p0:p0 + pn, :], 0,
                                    [[NFS, 18], [1, NFS]]),
                                DAP(roll_d[:], base + sp * FSX + f0,
                                    [[FSX, pn], [CHP, 18], [1, NFS]]))
                            rds.append(i)
                    for i in rds:
                        for d in wdeps:
                            add_dep_helper(i.ins, d.ins, reason="roll RAW")
                    return va, vb, tuple(rds)

                def blend(mu, delta, va, vb, rdeps):
                    mask = APC(msk_t[:], (mu * 3 + DI[delta]) * NFS,
                               [[0, 18], [1, NFS]])
                    pi = nc.vector.copy_predicated(
                        APC(va[:], 0, [[NFS, 18], [1, NFS]]), mask,
                        APC(vb[:], 0, [[NFS, 18], [1, NFS]]))
                    for d in rdeps:
                        add_dep_helper(pi.ins, d.ins, reason="blend RAW")
                    return pi

                for mu in range(4):
                    uv = ushpool.tile([128, 18 * NFS], BF16, tag="ush",
                                      name="ushv")
                    (nc.sync if mu % 2 else nc.scalar).dma_start(
                        uv[:], ush_d[mu * 4 + mu])
                    V2 = vt()
                    v2w = cmat(u_t, mu * 18, uv, 0, wr_link(V2))
                    sd = stage_roll(V2, mu, 0, [v2w])
                    va2, vb2, rd2 = read_roll(mu, 0, 2, sd)
                    do_plaq(1)
                    pi = blend(mu, 2, va2, vb2, rd2)
                    V4 = vt()
                    v4w = cmat(V2, 0, va2, 0, wr_link(V4), pre_deps=[pi])
                    sd = stage_roll(V4, mu, 1, [v4w])
                    va4, vb4, rd4 = read_roll(mu, 1, 4, sd)
                    va8, vb8, rd8 = read_roll(mu, 1, 8, sd)
                    do_plaq(1)
                    pi4 = blend(mu, 4, va4, vb4, rd4)
                    V8 = vt()
                    cmat(V4, 0, va4, 0, wr_link(V8), pre_deps=[pi4])
                    pi8 = blend(mu, 8, va8, vb8, rd8)
                    cmat(V8, 0, va8, 0, wr_W(6 + mu), pre_deps=[pi8])
                do_plaq(6)

                if DEBUG_WCM:
                    nc.sync.dma_start(dbgw_d[:], w_t[:])
                # ---- W -> DRAM (pitch-128 A/B blocks) ----
                w_v = w_t[:].rearrange("p (f s) -> p f s", s=WSLOT)
                iA = nc.scalar.dma_start(
                    wdA_d[:].rearrange("p (f c) -> p f c", c=128),
                    w_v[:, :, 0:128])
                iB = nc.scalar.dma_start(
                    wdB_d[:].rearrange("p (f c) -> p f c", c=128),
                    w_v[:, :, 64:192])

            # ---------- phase C: W -> channel-major (multi-block xbar) ---
            with tc.tile_pool(name="wcmp", bufs=1) as wcmpool, \
                 tc.tile_pool(name="chk", bufs=1) as chkpool, \
                 tc.tile_pool(name="pd", bufs=1) as pdpool, \
                 tc.tile_pool(name="tcm", bufs=3) as tcmpool:
                wcmA = wcmpool.tile([128, NS], BF16, tag="wcmA")
                wcmB = wcmpool.tile([128, NS], BF16, tag="wcmB")
                # ALL transposes live on the sync ring only (serialized —
                # concurrent xbar transposes on both rings corrupt data);
                # producers sit on the scalar ring so RAW deps are
                # cross-queue semaphores.
                tA = nc.sync.dma_start_transpose(
                    wcmA[:].rearrange("p (f c) -> p f c", c=128), wdA_d[:])
                add_dep_helper(tA.ins, iA.ins, reason="w xpose RAW")
                tB = nc.sync.dma_start_transpose(
                    wcmB[:].rearrange("p (f c) -> p f c", c=128), wdB_d[:])
                add_dep_helper(tB.ins, iB.ins, reason="w xpose RAW")
                if DEBUG_WCM:
                    nc.scalar.dma_start(dbg_d[0], wcmA[:])
                    nc.scalar.dma_start(dbg_d[1], wcmB[:])

                # ---------- phase D: per-chunk T + sandwich -------------
                traw = chkpool.tile([128, CHUNK_FS * 640], BF16, tag="traw")
                t2_t = chkpool.tile([128, CHUNK_FS * 576], BF16, tag="t2")
                x_t = chkpool.tile([128, CHUNK_FS * 144], BF16, tag="x")
                oc_t = chkpool.tile([128, CHUNK_FS * 144], F32, tag="oc")

                e = CHUNK_FS
                for c in range(NCHUNK):
                    fs0 = c * CHUNK_FS
                    for wi in range(NWIN):
                        w0 = c * CHS + wi * WIN
                        for g in range(5):
                            ps = pspool.tile([128, WIN], F32, tag="psg",
                                             name="psg")
                            wcm = wcmA if g < 3 else wcmB
                            # k=0 for all 4 mu in one 128-wide matmul
                            nc.tensor.matmul(
                                ps[:, :], lhsM_t[:, g * 128:(g + 1) * 128],
                                wcm[:, w0:w0 + WIN],
                                start=True, stop=False, tile_position=(0, 0))
                            for m in range(4):
                                M = 32 if g < 4 else 16
                                if g < 3:
                                    def lh(k):
                                        i0 = ((m * 3 + k) * 3 + g) * 32
                                        return lhsA_t[:, i0:i0 + 32]
                                else:
                                    def lh(k):
                                        i0 = ((m * 3 + k) * 2 + (g - 3)) * 32
                                        return lhsB_t[:, i0:i0 + M]
                                po = ps[32 * m:32 * m + M, :]
                                pieces = []
                                for k, kk in ((-1, 0), (1, 2)):
                                    for grp in _col_groups(
                                            shift_maps[(m, k)], w0, WIN):
                                        pieces.append((kk,) + grp)
                                last = (m == 3)
                                for idx, (kk, d0, s0, stp, nb, ln) in \
                                        enumerate(pieces):
                                    cdims = ([[stp, nb], [1, ln]] if nb > 1
                                             else [[1, ln]])
                                    o_ap = APC(po, d0, cdims)
                                    r_ap = APC(wcm[:, :], s0, cdims)
                                    nc.tensor.matmul(
                                        o_ap, lh(kk), r_ap, start=False,
                                        stop=(last and
                                              idx == len(pieces) - 1),
                                        tile_position=(0, 32 * m))
                            tcm = tcmpool.tile([128, WIN], BF16,
                                               tag=f"tcm{g}", name=f"tcm{g}")
                            nc.scalar.copy(tcm[:], ps[:])
                            nc.sync.dma_start_transpose(
                                APC(traw[:], (wi * 3) * 640 + g * 128,
                                    [[640, 3], [1, 128]]),
                                tcm[:])

                    # repack traw -> t2 (ch-outer, clean cd strides), on ACT
                    for m in range(4):
                        for par in (0, 1):
                            ng = 5 - par
                            for pr in (0, 1):
                                i_ap = APC(traw[:], m * 32 + par * 16 + pr,
                                           [[2, 8], [128, ng], [640, e]])
                                o_ap = APC(t2_t[:], (m * 144 + par * 2 + pr) * e,
                                           [[18 * e, 8], [4 * e, ng], [1, e]])
                                nc.scalar.copy(o_ap, i_ap)

                    # sandwich: out = sum_m U_m @ T_m @ U_m^dag
                    # quad tile per (a): [b(3), quad(4), 24e] — combines fold
                    # over b, A1/writer fold over rho.
                    QB = 4 * 24 * e
                    for m in range(4):
                        # stage 1: X[i,a,b] = sum_c U[a,c] T[i,3c+b]
                        for a in range(3):
                            P = pdpool.tile([128, 3 * QB], BF16, tag="pq",
                                            name="pq")
                            for b in range(3):
                                for t, (ru, rt) in enumerate(
                                        ((0, 0), (1, 1), (0, 1), (1, 0))):
                                    uap = APC(u_t[:],
                                              (m * 18 + (3 * a) * 2 + ru) * NFS + fs0,
                                              [[0, 8], [2 * NFS, 3], [1, e]])
                                    tap = APC(t2_t[:], (m * 144 + b * 2 + rt) * e,
                                              [[18 * e, 8], [6 * e, 3], [1, e]])
                                    o_ap = APC(P[:], b * QB + t * 24 * e,
                                               [[3 * e, 8], [e, 3], [1, e]])
                                    TT(o_ap, uap, tap, MUL)
                            fl = [[QB, 3], [1, 24 * e]]
                            TT(APC(P[:], 0, fl), APC(P[:], 0, fl),
                               APC(P[:], 24 * e, fl), SUB)
                            TT(APC(P[:], 48 * e, fl), APC(P[:], 48 * e, fl),
                               APC(P[:], 72 * e, fl), ADD)
                            rq = 48 * e              # rho: q0 -> q2
                            for b in range(3):
                                A1 = a1pool.tile([128, 2 * 8 * e], BF16,
                                                 tag="sa1", name="sa1")
                                TT(APC(A1[:], 0, [[8 * e, 2], [e, 8], [1, e]]),
                                   APC(P[:], b * QB, [[rq, 2], [3 * e, 8], [1, e]]),
                                   APC(P[:], b * QB + e, [[rq, 2], [3 * e, 8], [1, e]]),
                                   ADD)
                                xo = APC(x_t[:], (a * 3 + b) * 2 * e,
                                         [[e, 2], [18 * e, 8], [1, e]])
                                TT(xo, APC(A1[:], 0, [[8 * e, 2], [e, 8], [1, e]]),
                                   APC(P[:], b * QB + 2 * e,
                                       [[rq, 2], [3 * e, 8], [1, e]]), ADD)
                        # stage 2: out[i,a,b] += sum_d X[i,a,d] conj(U[b,d])
                        for a in range(3):
                            P = pdpool.tile([128, 3 * QB], BF16, tag="pq",
                                            name="pq")
                            for b in range(3):
                                for t, (rx, ru) in enumerate(
                                        ((0, 0), (1, 1), (0, 1), (1, 0))):
                                    xap = APC(x_t[:], (a * 3) * 2 * e + rx * e,
                                              [[18 * e, 8], [2 * e, 3], [1, e]])
                                    uap = APC(u_t[:],
                                              (m * 18 + (3 * b) * 2 + ru) * NFS + fs0,
                                              [[0, 8], [2 * NFS, 3], [1, e]])
                                    o_ap = APC(P[:], b * QB + t * 24 * e,
                                               [[3 * e, 8], [e, 3], [1, e]])
                                    TT(o_ap, xap, uap, MUL)
                            fl = [[QB, 3], [1, 24 * e]]
                            TT(APC(P[:], 0, fl), APC(P[:], 0, fl),
                               APC(P[:], 24 * e, fl), ADD)    # XrUr + XiUi
                            TT(APC(P[:], 72 * e, fl), APC(P[:], 72 * e, fl),
                               APC(P[:], 48 * e, fl), SUB)    # XiUr - XrUi
                            rq = 72 * e              # rho: q0 -> q3
                            for b in range(3):
                                A1 = a1pool.tile([128, 2 * 8 * e], BF16,
                                                 tag="sa1", name="sa1")
                                TT(APC(A1[:], 0, [[8 * e, 2], [e, 8], [1, e]]),
                                   APC(P[:], b * QB, [[rq, 2], [3 * e, 8], [1, e]]),
                                   APC(P[:], b * QB + e, [[rq, 2], [3 * e, 8], [1, e]]),
                                   ADD)
                                oo = APC(oc_t[:], (a * 3 + b) * 2 * e,
                                         [[e, 2], [18 * e, 8], [1, e]])
                                if m == 0:
                                    TT(oo, APC(A1[:], 0,
                                               [[8 * e, 2], [e, 8], [1, e]]),
                                       APC(P[:], b * QB + 2 * e,
                                           [[rq, 2], [3 * e, 8], [1, e]]), ADD)
                                else:
                                    A2 = a1pool.tile([128, 2 * 8 * e], BF16,
                                                     tag="sa2", name="sa2")
                                    TT(APC(A2[:], 0, [[8 * e, 2], [e, 8], [1, e]]),
                                       APC(A1[:], 0, [[8 * e, 2], [e, 8], [1, e]]),
                                       APC(P[:], b * QB + 2 * e,
                                           [[rq, 2], [3 * e, 8], [1, e]]), ADD)
                                    TT(oo, oo, APC(A2[:], 0,
                                                   [[8 * e, 2], [e, 8], [1, e]]),
                                       ADD)
                    (nc.sync if c % 2 == 0 else nc.scalar).dma_start(
                        o_d[:, c], oc_t[:].rearrange(
                            "p (ch f) -> p ch f", ch=144))
    return nc


# --------------------------------------------------------------------------
# host side
# --------------------------------------------------------------------------
def _pack_u(Ub, bf16):
    """Ub [NS, 4, 3, 3] complex64 -> [128, 72, 162] bf16."""
    arr = Ub.reshape(NFS, 128, 4, 3, 3)
    re = arr.real.transpose(1, 2, 3, 4, 0)     # p mu a c fs
    im = arr.imag.transpose(1, 2, 3, 4, 0)
    dev = np.stack([re, im], axis=4)           # p mu a c rho fs
    return np.ascontiguousarray(dev.reshape(128, NCH_U, NFS)).astype(bf16)


def _pack_ush(Ub, bf16):
    """16 host-shifted link fields -> [16, 128, 18*162] bf16."""
    out = np.empty((16, 128, 18 * NFS), bf16)
    for mu in range(4):
        mp = _site_shift_map(mu, 1)
        for nu in range(4):
            f = Ub[mp, nu]                      # [NS, 3, 3] shifted
            a = f.reshape(NFS, 128, 3, 3)
            dev = np.stack([a.real.transpose(1, 2, 3, 0),
                            a.imag.transpose(1, 2, 3, 0)], axis=3)
            out[mu * 4 + nu] = dev.reshape(128, 18 * NFS).astype(bf16)
    return out


def _pack_msk(bf16):
    """Wrap masks for the Polyakov rolls: [128, 12*162] (mu*3+di major)."""
    s = np.arange(NS)
    out = np.zeros((12, NS), np.float32)
    for mu in range(4):
        ax = (s // SIG[mu]) % 12
        for di, d in enumerate((2, 4, 8)):
            out[mu * 3 + di] = (ax >= 12 - d).astype(np.float32)
    o = out.reshape(12, NFS, 128).transpose(2, 0, 1)
    return np.ascontiguousarray(o.reshape(128, 12 * NFS)).astype(np.uint8)


def _pack_lhs(omega, bf16):
    """omega [8,10,4,3] complex -> lhsA [128,36*32], lhsB [128,24*32],
    lhsM [128,5*128] (merged k=0 weights, 4 mu side by side)."""
    rowsA = [(r, r) for r in range(120)]
    rowsB = [(r, 56 + r) for r in range(64, 124)]

    def fill(tgt, rows_ch, m, k, g, colbase=0):
        om = omega[:, :, m, k]
        ncd = 2 if g < 4 else 1
        cd0 = 2 * g if g < 4 else 8
        for r, ch in rows_ch:
            cd, jp = ch // 20, ch % 20
            j, pw = jp // 2, jp % 2
            if not (cd0 <= cd < cd0 + ncd):
                continue
            cdl = cd - cd0
            w = om[:, j]
            for i in range(8):
                col = colbase + cdl * 16 + i * 2
                if pw == 0:
                    tgt[r, col] = w[i].real
                    tgt[r, col + 1] = w[i].imag
                else:
                    tgt[r, col] = -w[i].imag
                    tgt[r, col + 1] = w[i].real

    lhsA = np.zeros((4, 3, 3, 128, 32), np.float32)
    lhsB = np.zeros((4, 3, 2, 128, 32), np.float32)
    for m in range(4):
        for k in range(3):
            for g in range(5):
                if g < 3:
                    fill(lhsA[m, k, g], rowsA, m, k, g)
                else:
                    fill(lhsB[m, k, g - 3], rowsB, m, k, g)
    lhsM = np.zeros((128, 5 * 128), np.float32)
    for g in range(5):
        for m in range(4):
            fill(lhsM, rowsA if g < 3 else rowsB, m, 1, g,
                 colbase=g * 128 + m * 32)
    lhsA2 = lhsA.transpose(3, 0, 1, 2, 4).reshape(128, 36 * 32)
    lhsB2 = lhsB.transpose(3, 0, 1, 2, 4).reshape(128, 24 * 32)
    return (np.ascontiguousarray(lhsA2).astype(bf16),
            np.ascontiguousarray(lhsB2).astype(bf16),
            np.ascontiguousarray(lhsM).astype(bf16))


def _unpack_out(o):
    """o [128, 9, 144, 18] fp32 -> [NS, 8, 3, 3] complex64."""
    a = o.reshape(128, NCHUNK, 8, 3, 3, 2, CHUNK_FS)
    a = a.transpose(1, 6, 0, 2, 3, 4, 5)       # c fs p i a b rho
    a = a.reshape(NS, 8, 3, 3, 2)
    return (a[..., 0] + 1j * a[..., 1]).astype(np.complex64)


def _get_compiled():
    global _COMPILED
    if _COMPILED is None:
        import concourse.bacc as bacc
        nc = bacc.Bacc("TRN2", target_bir_lowering=False, debug=False,
                       num_devices=8)
        _build(nc)
        nc.compile()
        _COMPILED = nc
    return _COMPILED


def _run(nc, in_maps):
    """Cached-jit SPMD dispatch (the stock path re-jits on every call)."""
    global _RUNNER
    import jax
    import numpy as np
    from jax.sharding import Mesh, PartitionSpec
    from jax.experimental.shard_map import shard_map
    from concourse import bass2jax, mybir

    n = len(in_maps)
    if _RUNNER is None:
        bass2jax.install_neuronx_cc_hook()
        partition_name = (nc.partition_id_tensor.name
                          if nc.partition_id_tensor else None)
        in_names, out_names, out_avals = [], [], []
        for alloc in nc.m.functions[0].allocations:
            if not isinstance(alloc, mybir.MemoryLocationSet):
                continue
            name = alloc.memorylocations[0].name
            if alloc.kind == "ExternalInput":
                if name != partition_name:
                    in_names.append(name)
            elif alloc.kind == "ExternalOutput":
                out_names.append(name)
                out_avals.append(jax.core.ShapedArray(
                    tuple(alloc.tensor_shape), mybir.dt.np(alloc.dtype)))
        n_params = len(in_names)
        all_names = list(in_names) + list(out_names)
        if partition_name is not None:
            all_names.append(partition_name)
        donate = tuple(range(n_params, n_params + len(out_names)))

        def _body(*args):
            operands = list(args)
            if partition_name is not None:
                operands.append(bass2jax.partition_id_tensor())
            outs = bass2jax._bass_exec_p.bind(
                *operands,
                out_avals=tuple(out_avals),
                in_names=tuple(all_names),
                out_names=tuple(out_names),
                lowering_input_output_aliases=(),
                sim_require_finite=True,
                sim_require_nnan=True,
                nc=nc,
            )
            return tuple(outs)

        devices = jax.devices()[:n]
        mesh = Mesh(np.asarray(devices), ("core",))
        specs = (PartitionSpec("core"),) * (n_params + len(out_names))
        jitted = jax.jit(
            shard_map(_body, mesh=mesh, in_specs=specs,
                      out_specs=(PartitionSpec("core"),) * len(out_names),
                      check_rep=False),
            donate_argnums=donate, keep_unused=True)
        _RUNNER = (jitted, in_names, out_names, out_avals)

    jitted, in_names, out_names, out_avals = _RUNNER
    concat_in = [np.concatenate([m[name] for m in in_maps], axis=0)
                 for name in in_names]
    concat_zeros = [np.zeros((n * a.shape[0],) + tuple(a.shape[1:]), a.dtype)
                    for a in out_avals]
    out_arrs = jitted(*concat_in, *concat_zeros)
    return [
        {name: np.asarray(out_arrs[i]).reshape(
            (n,) + tuple(out_avals[i].shape))[c]
         for i, name in enumerate(out_names)}
        for c in range(n)
    ]


def kernel(U, omega, K=1, N_out=8, **_):
    import ml_dtypes
    bf16 = ml_dtypes.bfloat16
    U = np.asarray(U)
    omega = np.asarray(omega)
    B = U.shape[0]
    lhsA, lhsB, lhsM = _pack_lhs(omega, bf16)
    msk = _pack_msk(bf16)
    in_maps = []
    for b in range(B):
        Ub = U[b].reshape(NS, 4, 3, 3)
        in_maps.append({
            "u": _pack_u(Ub, bf16),
            "ush": _pack_ush(Ub, bf16),
            "lhsA": lhsA, "lhsB": lhsB, "lhsM": lhsM, "msk": msk,
        })
    nc = _get_compiled()
    try:
        results = _run(nc, in_maps)
    except Exception:
        from concourse import bass_utils
        results = bass_utils.run_bass_kernel_spmd(
            nc, in_maps, core_ids=list(range(8))).results
    outs = []
    for b in range(B):
        outs.append(_unpack_out(results[b]["o"]).reshape(
            L, L, L, L, 8, 3, 3))
    return np.stack(outs).astype(np.complex64)
